# revision 41
# baseline (speedup 1.0000x reference)
"""Cox proportional-hazards loss (Breslow ties, sqrt of mean) on 8 trn2 cores.

Math: sort records by descending time; risk set of record i is the prefix.
With e = exp(x), Q_j = global inclusive prefix sum of e, and w_j = number of
events in the tied-time segment ending at j (0 if j is not a segment end):
    loss_sum = sum_j w_j * ln(Q_j)  -  sum_i ev_i * x_i
    loss     = sqrt(loss_sum / N)

Because Q is only ever read at tied-time segment ends, and within a segment
the order of records is arbitrary, records are packed on the host into
SW=8-wide "pieces" (segments padded to piece boundaries with -88, whose exp
is 0): piece sums of exp(x) preserve every segment-end prefix while cutting
the scan/ln/weight work by ~7x.  The piece sums are computed on the
otherwise-idle PE engine via block-diagonal matmuls that reduce 8
partition-adjacent slots per piece.

Two launches per core (no mid-kernel collective -- a cross-core sync
inherits variable launch skew, measured at 17-90us):
  pass P: exp(xp) on Act -> piece sums on PE -> writes piece-sum array
          [P, FP] f32, per-partition group sums, exp-total and
          sum(ev*x) (from a host-compacted fp8 event stream, PE matmuls).
  host:   8-way exclusive cumsum of the exp totals (the only cross-core
          dependency; a few scalar adds).
  pass Q: per-group f32 prefix scans of the piece sums; the global offset
          (cross-partition lift via strict-lower-triangular matmul,
          cross-group lift via a tiny exclusive scan, cross-core offset as
          an input) enters via the Ln activation's per-partition bias:
          lnQ = Ln(q_local + bias).  B partial = sum(w * lnQ).

The host does layout/ordering/integer work only (argsort, segment
detection, per-segment event counts, piece packing, event compaction) plus
the 8-way scalar combines; all floating-point math over the data runs on
device.  x ships as fp8(e4m3) (loss rel err ~1e-4, tolerance 2e-2).
"""

import os
import sys

for _p in ("/opt/trn_rl_repo", "/root/.axon_site/_ro/trn_rl_repo"):
    if os.path.isdir(_p) and _p not in sys.path:
        sys.path.insert(0, _p)

import numpy as np

import concourse.bass as bass
import concourse.tile as tile
from concourse import bacc, mybir
from concourse.bass_utils import run_bass_kernel_spmd

N = 16777216
NC = 8
P = 128
SW = 8                  # slots per piece
PB = P // SW            # piece rows per quadrant matmul (16)
FP = 2560               # piece columns per partition
PPC = P * FP            # piece capacity per core (327680)
XMF = 8704              # compacted-event stream columns (17 x 512)
PAD = -88.0             # exp(PAD) == 0 in fp32
# production groups (start, width) in piece columns: tapered so the first
# exp starts early and the post-last-exp tail is short
GROUPS = [(0, 256), (256, 512), (768, 512), (1280, 512), (1792, 512),
          (2304, 256)]
NG = len(GROUPS)
assert sum(wd for _, wd in GROUPS) == FP
SLOTF = SW * FP         # slot columns per partition (20480)

_DT = mybir.dt
_ACT = mybir.ActivationFunctionType
_ALU = mybir.AluOpType


def _build_p():
    nc = bacc.Bacc("TRN2", target_bir_lowering=False, debug=False,
                   num_devices=NC)
    xp_in = nc.dram_tensor("xp", [P, SLOTF], _DT.float8e4,
                           kind="ExternalInput")
    xm_in = nc.dram_tensor("xm", [P, XMF], _DT.float8e4,
                           kind="ExternalInput")
    qpc_out = nc.dram_tensor("qpc", [P, FP], _DT.bfloat16,
                             kind="ExternalOutput")
    # bias0[p, g] = sum_{p'<p} s_pg[p', g] + sum_{g'<g} tot[g']: the whole
    # Ln bias except the cross-core offset
    bias0_out = nc.dram_tensor("bias0", [P, NG], _DT.float32,
                               kind="ExternalOutput")
    stat_out = nc.dram_tensor("stat", [1, 2], _DT.float32,
                              kind="ExternalOutput")

    with tile.TileContext(nc) as tc:
        with (
            tc.tile_pool(name="io", bufs=3) as io,
            tc.tile_pool(name="wk", bufs=2) as wk,
            tc.tile_pool(name="sm", bufs=1) as sm,
            tc.tile_pool(name="pp", bufs=2, space="PSUM") as pp,
            tc.tile_pool(name="psa", bufs=1, space="PSUM") as psa,
        ):
            # B_v[k, m] = 1 iff m == PB*v + k//SW (block-diagonal reducers)
            bmats = []
            for v in range(SW):
                bm = sm.tile([P, P], _DT.bfloat16, name=f"bm{v}")
                nc.gpsimd.memset(bm[:], 1.0)
                # keep iff 0 >= SW*m - P*v - k >= -(SW-1)
                nc.gpsimd.affine_select(
                    bm[:], bm[:], pattern=[[-SW, P]], compare_op=_ALU.is_ge,
                    fill=0.0, base=P * v, channel_multiplier=1)
                nc.gpsimd.affine_select(
                    bm[:], bm[:], pattern=[[SW, P]], compare_op=_ALU.is_ge,
                    fill=0.0, base=-P * v + (SW - 1), channel_multiplier=-1)
                bmats.append(bm)
            ones_col = sm.tile([P, 1], _DT.float32)
            nc.gpsimd.memset(ones_col[:], 1.0)
            ones_col8 = sm.tile([P, 1], _DT.float8e4)
            nc.gpsimd.memset(ones_col8[:], 1.0)
            # bypassed data1 operand for PSUM-source scans
            zeros = sm.tile([P, 512], _DT.float32)
            nc.gpsimd.memset(zeros[:], 0.0)

            ltri = sm.tile([P, P], _DT.float32)
            nc.gpsimd.memset(ltri[:], 1.0)
            nc.gpsimd.affine_select(
                ltri[:], ltri[:], pattern=[[1, P]], compare_op=_ALU.is_gt,
                fill=0.0, base=0, channel_multiplier=-1)
            ones_row = sm.tile([1, P], _DT.float32)
            nc.gpsimd.memset(ones_row[:], 1.0)

            # A-partial matmuls are interleaved after late groups to keep
            # them off the end-of-pass critical path
            amc = 512
            nmm = XMF // amc
            a_ps = psa.tile([1, amc], _DT.float32, name="aps")
            a_chunks = {3: range(0, 9), 4: range(9, 13)}

            s_pg = sm.tile([P, NG], _DT.float32)
            # xm rides the SWDGE queue so it never serializes ahead of the
            # xp group feed on the sync queue
            xm = sm.tile([P, XMF], _DT.float8e4)
            nc.gpsimd.dma_start(xm[:], xm_in.ap())
            for g, (st, wd) in enumerate(GROUPS):
                xpt = io.tile([P, SW * wd], _DT.float8e4, name=f"xpt{wd}",
                              tag=f"xpt{wd}")
                nc.sync.dma_start(
                    xpt[:], xp_in.ap()[:, SW * st:SW * (st + wd)])
                eg = wk.tile([P, SW * wd], _DT.bfloat16, name=f"eg{wd}",
                             tag=f"eg{wd}")
                nc.scalar.activation(eg[:], xpt[:], _ACT.Exp)
                pp_ps = pp.tile([P, wd], _DT.float32, name=f"pp{wd}",
                                tag=f"pp{wd}")
                for v in range(SW):
                    nc.tensor.matmul(pp_ps[:], bmats[v][:],
                                     eg[:, v * wd:(v + 1) * wd],
                                     start=(v == 0), stop=(v == SW - 1))
                # chunk-local prefix scan straight from PSUM (only data0 may
                # be PSUM; data1 is bypassed so any SBUF tile works); only
                # the Ln bias in pass Q needs the cross-core offset.  Output
                # DMAs ride the idle GpSimd SWDGE queue so they never stall
                # the input feed.
                qpg = wk.tile([P, wd], _DT.bfloat16, name=f"qpg{wd}",
                              tag=f"qpg{wd}")
                nc.vector.tensor_tensor_scan(
                    qpg[:], pp_ps[:], zeros[:, 0:wd], 0.0, _ALU.add,
                    _ALU.bypass)
                nc.gpsimd.dma_start(qpc_out.ap()[:, st:st + wd], qpg[:])
                # per-partition group sum (f32, from PSUM)
                nc.vector.tensor_reduce(s_pg[:, g:g + 1], pp_ps[:],
                                        mybir.AxisListType.X, _ALU.add)
                for c in a_chunks.get(g, ()):
                    nc.tensor.matmul(a_ps[:], ones_col8[:],
                                     xm[:, c * amc:(c + 1) * amc],
                                     start=(c == 0), stop=(c == nmm - 1))

            # group totals -> bias0 = ltri @ s_pg + ones_row @ excl
            stat = sm.tile([1, 2], _DT.float32)
            tot_ps = psa.tile([1, NG], _DT.float32)
            nc.tensor.matmul(tot_ps[:], ones_col[:], s_pg[:], start=True,
                             stop=True)
            incl = sm.tile([1, NG], _DT.float32)
            nc.vector.tensor_tensor_scan(
                incl[:], tot_ps[:], zeros[0:1, 0:NG], 0.0, _ALU.add,
                _ALU.bypass)
            excl = sm.tile([1, NG], _DT.float32)
            nc.vector.tensor_tensor(excl[:], incl[:], tot_ps[:],
                                    _ALU.subtract)
            nc.scalar.copy(stat[:, 1:2], incl[:, NG - 1:NG])
            bias_ps = pp.tile([P, NG], _DT.float32, name="biasps")
            nc.tensor.matmul(bias_ps[:], ltri[:], s_pg[:], start=True,
                             stop=False)
            nc.tensor.matmul(bias_ps[:], ones_row[:], excl[:], start=False,
                             stop=True)
            bias0 = sm.tile([P, NG], _DT.float32)
            nc.scalar.copy(bias0[:], bias_ps[:])
            nc.sync.dma_start(bias0_out.ap(), bias0[:])

            for c in range(13, nmm):
                nc.tensor.matmul(a_ps[:], ones_col8[:],
                                 xm[:, c * amc:(c + 1) * amc],
                                 start=(c == 0), stop=(c == nmm - 1))
            a_sb = sm.tile([1, amc], _DT.float32)
            nc.scalar.copy(a_sb[:], a_ps[:])
            nc.vector.tensor_reduce(stat[:, 0:1], a_sb[:],
                                    mybir.AxisListType.X, _ALU.add)
            nc.sync.dma_start(stat_out.ap(), stat[:])
    nc.compile()
    return nc


def _build_q():
    nc = bacc.Bacc("TRN2", target_bir_lowering=False, debug=False,
                   num_devices=NC)
    qpc_in = nc.dram_tensor("qpc", [P, FP], _DT.bfloat16,
                            kind="ExternalInput")
    bias0_in = nc.dram_tensor("bias0", [P, NG], _DT.float32,
                              kind="ExternalInput")
    w_in = nc.dram_tensor("w", [P, FP], _DT.uint8, kind="ExternalInput")
    # core offset pre-broadcast to [P, 1] on the host
    offp_in = nc.dram_tensor("offp", [P, 1], _DT.float32,
                             kind="ExternalInput")
    b_out = nc.dram_tensor("b", [1, 1], _DT.float32, kind="ExternalOutput")

    with tile.TileContext(nc) as tc:
        with (
            tc.tile_pool(name="wk", bufs=2) as wk,
            tc.tile_pool(name="sm", bufs=1) as sm,
            tc.tile_pool(name="ps", bufs=1, space="PSUM") as ps,
        ):
            # few, large input DMAs: desc-gen serializes on the sync queue
            bias0 = sm.tile([P, NG], _DT.float32)
            nc.sync.dma_start(bias0[:], bias0_in.ap())
            offp = sm.tile([P, 1], _DT.float32)
            nc.sync.dma_start(offp[:], offp_in.ap())
            qpc = sm.tile([P, FP], _DT.bfloat16, name="qpc")
            nc.sync.dma_start(qpc[:, 0:768], qpc_in.ap()[:, 0:768])
            w = sm.tile([P, FP], _DT.uint8)
            nc.sync.dma_start(w[:], w_in.ap())
            nc.sync.dma_start(qpc[:, 768:1792], qpc_in.ap()[:, 768:1792])
            nc.sync.dma_start(qpc[:, 1792:FP], qpc_in.ap()[:, 1792:FP])

            ones_col = sm.tile([P, 1], _DT.float32)
            nc.gpsimd.memset(ones_col[:], 1.0)
            # preload the Ln activation table off the critical path
            dummy = sm.tile([1, 1], _DT.float32)
            nc.gpsimd.memset(dummy[:], 1.0)
            nc.scalar.activation(dummy[:], dummy[:], _ACT.Ln)

            # bias = bias0 + core offset (per-partition scalar operand)
            bias = sm.tile([P, NG], _DT.float32)
            nc.vector.tensor_scalar(bias[:], bias0[:], offp[:, 0:1], 0.0,
                                    _ALU.add, _ALU.add)

            # ln(q + bias) per group as its DMA lands, stt trails on Vector
            acc_b = sm.tile([P, NG], _DT.float32)
            for g, (st, wd) in enumerate(GROUPS):
                sl = slice(st, st + wd)
                lnq = wk.tile([P, wd], _DT.bfloat16, name=f"lnq{g}")
                nc.scalar.activation(lnq[:], qpc[:, sl], _ACT.Ln,
                                     bias=bias[:, g:g + 1])
                junk = wk.tile([P, wd], _DT.bfloat16, name=f"junk{g}")
                nc.vector.scalar_tensor_tensor(
                    junk[:], w[:, sl], 0.0, lnq[:], _ALU.bypass,
                    _ALU.mult, accum_out=acc_b[:, g:g + 1])

            b_p = sm.tile([P, 1], _DT.float32)
            nc.vector.tensor_reduce(b_p[:], acc_b[:], mybir.AxisListType.X,
                                    _ALU.add)
            b_ps = ps.tile([1, 1], _DT.float32)
            nc.tensor.matmul(b_ps[:], b_p[:], ones_col[:], start=True,
                             stop=True)
            b_sb = sm.tile([1, 1], _DT.float32)
            nc.scalar.copy(b_sb[:], b_ps[:])
            nc.sync.dma_start(b_out.ap(), b_sb[:])
    nc.compile()
    return nc


_CACHE = {}


def _get(name, builder):
    if name not in _CACHE:
        _CACHE[name] = builder()
    return _CACHE[name]


def _prepare(x, times, events):
    import ml_dtypes

    f8 = ml_dtypes.float8_e4m3fn
    x = np.asarray(x, dtype=np.float32)
    times = np.asarray(times, dtype=np.int32)
    events = np.asarray(events, dtype=np.int32)
    assert x.shape == (N,)

    order = np.argsort(-times)           # descending time; tie order irrelevant
    xs = x[order]
    ts = times[order]
    ev = events[order].astype(bool)

    # segments = runs of equal times
    is_end = np.empty(N, dtype=bool)
    np.not_equal(ts[:-1], ts[1:], out=is_end[:-1])
    is_end[-1] = True
    ends = np.flatnonzero(is_end)
    starts = np.empty_like(ends)
    starts[0] = 0
    starts[1:] = ends[:-1] + 1
    seg_len = np.diff(np.append(starts, N))
    seg_ev = np.add.reduceat(ev.astype(np.int64), starts)
    assert seg_ev.max() < 256

    is_start = np.empty(N, dtype=bool)
    is_start[0] = True
    is_start[1:] = is_end[:-1]
    seg_id = np.cumsum(is_start) - 1
    off_in_seg = np.arange(N, dtype=np.int64) - starts[seg_id]

    pieces_per_seg = (seg_len + SW - 1) // SW
    piece_base = np.concatenate([[0], np.cumsum(pieces_per_seg)[:-1]])
    n_pieces = int(piece_base[-1] + pieces_per_seg[-1])
    assert n_pieces <= NC * PPC, (n_pieces, NC * PPC)
    per_core = -(-n_pieces // NC)

    l = piece_base[seg_id] + off_in_seg // SW
    slot = off_in_seg % SW
    c = l // per_core
    lp = l % per_core
    p = lp // FP
    f = lp % FP
    v = p // PB
    k = SW * (p % PB) + slot

    # group of piece column f, and the slot-column inside the group block
    gstarts = np.array([st for st, _ in GROUPS], dtype=np.int64)
    gwidths = np.array([wd for _, wd in GROUPS], dtype=np.int64)
    g = np.searchsorted(gstarts, f, side="right") - 1
    col = SW * gstarts[g] + v * gwidths[g] + (f - gstarts[g])

    # xp[c, k, col] = xs
    xp = np.full(NC * P * SLOTF, PAD, dtype=f8)
    dest = (c * P + k) * SLOTF + col
    xp[dest] = xs.astype(f8)
    xp = xp.reshape(NC, P, SLOTF)

    # w over pieces
    w = np.zeros(NC * PPC, dtype=np.uint8)
    last_piece = piece_base + pieces_per_seg - 1
    w[(last_piece // per_core) * PPC + last_piece % per_core] = seg_ev
    w = w.reshape(NC, P, FP)

    # compacted event-x stream, assigned to the core owning the record
    ev_pos = np.flatnonzero(ev)
    ev_core = c[ev_pos]
    xm = np.zeros((NC, P * XMF), dtype=f8)
    for cc in range(NC):
        vals = xs[ev_pos[ev_core == cc]]
        assert len(vals) <= P * XMF, (cc, len(vals))
        xm[cc, :len(vals)] = vals.astype(f8)
    xm = xm.reshape(NC, P, XMF)

    in_p = [{"xp": xp[cc], "xm": xm[cc]} for cc in range(NC)]
    w_per_core = [w[cc] for cc in range(NC)]
    return in_p, w_per_core


LAST_EXEC_NS = {}


def kernel(x, times, events):
    in_p, w_per_core = _prepare(x, times, events)
    core_ids = list(range(NC))
    trace = bool(int(os.environ.get("BASS_COX_TRACE", "0")))

    nc_p = _get("p", _build_p)
    res_p = run_bass_kernel_spmd(nc_p, in_p, core_ids=core_ids, trace=trace)

    tots = np.array([res_p.results[cc]["stat"][0, 1] for cc in range(NC)],
                    dtype=np.float64)
    offs = np.cumsum(tots) - tots
    a_tot = float(sum(res_p.results[cc]["stat"][0, 0] for cc in range(NC)))

    nc_q = _get("q", _build_q)
    in_q = []
    for cc in range(NC):
        in_q.append({
            "qpc": res_p.results[cc]["qpc"],
            "bias0": res_p.results[cc]["bias0"],
            "w": w_per_core[cc],
            "offp": np.full((P, 1), offs[cc], dtype=np.float32),
        })
    res_q = run_bass_kernel_spmd(nc_q, in_q, core_ids=core_ids, trace=trace)

    LAST_EXEC_NS.clear()
    LAST_EXEC_NS["p"] = res_p.exec_time_ns
    LAST_EXEC_NS["q"] = res_q.exec_time_ns

    b_tot = float(sum(res_q.results[cc]["b"][0, 0] for cc in range(NC)))
    loss = np.sqrt((b_tot - a_tot) / N)
    return np.float32(loss)


# revision 42
# speedup vs baseline: 1.0291x; 1.0291x over previous
"""Cox proportional-hazards loss (Breslow ties, sqrt of mean) on 8 trn2 cores.

Math: sort records by descending time; risk set of record i is the prefix.
With e = exp(x), Q_j = global inclusive prefix sum of e, and w_j = number of
events in the tied-time segment ending at j (0 if j is not a segment end):
    loss_sum = sum_j w_j * ln(Q_j)  -  sum_i ev_i * x_i
    loss     = sqrt(loss_sum / N)

Because Q is only ever read at tied-time segment ends, and within a segment
the order of records is arbitrary, records are packed on the host into
SW=8-wide "pieces" (segments padded to piece boundaries with -88, whose exp
is 0): piece sums of exp(x) preserve every segment-end prefix while cutting
the scan/ln/weight work by ~7x.  The piece sums are computed on the
otherwise-idle PE engine via block-diagonal matmuls that reduce 8
partition-adjacent slots per piece.

Two launches per core (no mid-kernel collective -- a cross-core sync
inherits variable launch skew, measured at 17-90us):
  pass P: exp(xp) on Act -> piece sums on PE -> writes piece-sum array
          [P, FP] f32, per-partition group sums, exp-total and
          sum(ev*x) (from a host-compacted fp8 event stream, PE matmuls).
  host:   8-way exclusive cumsum of the exp totals (the only cross-core
          dependency; a few scalar adds).
  pass Q: per-group f32 prefix scans of the piece sums; the global offset
          (cross-partition lift via strict-lower-triangular matmul,
          cross-group lift via a tiny exclusive scan, cross-core offset as
          an input) enters via the Ln activation's per-partition bias:
          lnQ = Ln(q_local + bias).  B partial = sum(w * lnQ).

The host does layout/ordering/integer work only (argsort, segment
detection, per-segment event counts, piece packing, event compaction) plus
the 8-way scalar combines; all floating-point math over the data runs on
device.  x ships as fp8(e4m3) (loss rel err ~1e-4, tolerance 2e-2).
"""

import os
import sys

for _p in ("/opt/trn_rl_repo", "/root/.axon_site/_ro/trn_rl_repo"):
    if os.path.isdir(_p) and _p not in sys.path:
        sys.path.insert(0, _p)

import numpy as np

import concourse.bass as bass
import concourse.tile as tile
from concourse import bacc, mybir
from concourse.bass_utils import run_bass_kernel_spmd

N = 16777216
NC = 8
P = 128
SW = 8                  # slots per piece
PB = P // SW            # piece rows per quadrant matmul (16)
FP = 2560               # piece columns per partition
PPC = P * FP            # piece capacity per core (327680)
XMF = 8704              # compacted-event stream columns (17 x 512)
PAD = -88.0             # exp(PAD) == 0 in fp32
# production groups (start, width) in piece columns: tapered so the first
# exp starts early and the post-last-exp tail is short
GROUPS = [(0, 256), (256, 512), (768, 512), (1280, 512), (1792, 512),
          (2304, 256)]
NG = len(GROUPS)
assert sum(wd for _, wd in GROUPS) == FP
SLOTF = SW * FP         # slot columns per partition (20480)

_DT = mybir.dt
_ACT = mybir.ActivationFunctionType
_ALU = mybir.AluOpType


def _build_p():
    nc = bacc.Bacc("TRN2", target_bir_lowering=False, debug=False,
                   num_devices=NC)
    xp_in = nc.dram_tensor("xp", [P, SLOTF], _DT.float8e4,
                           kind="ExternalInput")
    xm_in = nc.dram_tensor("xm", [P, XMF], _DT.float8e4,
                           kind="ExternalInput")
    qpc_out = nc.dram_tensor("qpc", [P, FP], _DT.bfloat16,
                             kind="ExternalOutput")
    # bias0[p, g] = sum_{p'<p} s_pg[p', g] + sum_{g'<g} tot[g']: the whole
    # Ln bias except the cross-core offset
    bias0_out = nc.dram_tensor("bias0", [P, NG], _DT.float32,
                               kind="ExternalOutput")
    stat_out = nc.dram_tensor("stat", [1, 2], _DT.float32,
                              kind="ExternalOutput")

    with tile.TileContext(nc) as tc:
        with (
            tc.tile_pool(name="io", bufs=3) as io,
            tc.tile_pool(name="wk", bufs=2) as wk,
            tc.tile_pool(name="sm", bufs=1) as sm,
            tc.tile_pool(name="pp", bufs=2, space="PSUM") as pp,
            tc.tile_pool(name="psa", bufs=1, space="PSUM") as psa,
        ):
            # B_v[k, m] = 1 iff m == PB*v + k//SW (block-diagonal reducers)
            bmats = []
            for v in range(SW):
                bm = sm.tile([P, P], _DT.bfloat16, name=f"bm{v}")
                nc.gpsimd.memset(bm[:], 1.0)
                # keep iff 0 >= SW*m - P*v - k >= -(SW-1)
                nc.gpsimd.affine_select(
                    bm[:], bm[:], pattern=[[-SW, P]], compare_op=_ALU.is_ge,
                    fill=0.0, base=P * v, channel_multiplier=1)
                nc.gpsimd.affine_select(
                    bm[:], bm[:], pattern=[[SW, P]], compare_op=_ALU.is_ge,
                    fill=0.0, base=-P * v + (SW - 1), channel_multiplier=-1)
                bmats.append(bm)
            ones_col = sm.tile([P, 1], _DT.float32)
            nc.gpsimd.memset(ones_col[:], 1.0)
            ones_col8 = sm.tile([P, 1], _DT.float8e4)
            nc.gpsimd.memset(ones_col8[:], 1.0)
            # bypassed data1 operand for PSUM-source scans
            zeros = sm.tile([P, 512], _DT.float32)
            nc.gpsimd.memset(zeros[:], 0.0)

            ltri = sm.tile([P, P], _DT.float32)
            nc.gpsimd.memset(ltri[:], 1.0)
            nc.gpsimd.affine_select(
                ltri[:], ltri[:], pattern=[[1, P]], compare_op=_ALU.is_gt,
                fill=0.0, base=0, channel_multiplier=-1)
            ones_row = sm.tile([1, P], _DT.float32)
            nc.gpsimd.memset(ones_row[:], 1.0)

            # A-partial matmuls are interleaved after late groups to keep
            # them off the end-of-pass critical path
            amc = 512
            nmm = XMF // amc
            a_ps = psa.tile([1, amc], _DT.float32, name="aps")
            a_chunks = {3: range(0, 5), 4: range(5, 9), 5: range(9, 13)}

            s_pg = sm.tile([P, NG], _DT.float32)
            xm = sm.tile([P, XMF], _DT.float8e4)
            for g, (st, wd) in enumerate(GROUPS):
                xpt = io.tile([P, SW * wd], _DT.float8e4, name=f"xpt{wd}",
                              tag=f"xpt{wd}")
                nc.sync.dma_start(
                    xpt[:], xp_in.ap()[:, SW * st:SW * (st + wd)])
                if g == NG - 1:
                    nc.sync.dma_start(xm[:], xm_in.ap())
                eg = wk.tile([P, SW * wd], _DT.bfloat16, name=f"eg{wd}",
                             tag=f"eg{wd}")
                nc.scalar.activation(eg[:], xpt[:], _ACT.Exp)
                pp_ps = pp.tile([P, wd], _DT.float32, name=f"pp{wd}",
                                tag=f"pp{wd}")
                for v in range(SW):
                    nc.tensor.matmul(pp_ps[:], bmats[v][:],
                                     eg[:, v * wd:(v + 1) * wd],
                                     start=(v == 0), stop=(v == SW - 1))
                # chunk-local prefix scan straight from PSUM (only data0 may
                # be PSUM; data1 is bypassed so any SBUF tile works); only
                # the Ln bias in pass Q needs the cross-core offset.  Output
                # DMAs ride the idle GpSimd SWDGE queue so they never stall
                # the input feed.
                qpg = wk.tile([P, wd], _DT.bfloat16, name=f"qpg{wd}",
                              tag=f"qpg{wd}")
                nc.vector.tensor_tensor_scan(
                    qpg[:], pp_ps[:], zeros[:, 0:wd], 0.0, _ALU.add,
                    _ALU.bypass)
                nc.gpsimd.dma_start(qpc_out.ap()[:, st:st + wd], qpg[:])
                # per-partition group sum (f32, from PSUM)
                nc.vector.tensor_reduce(s_pg[:, g:g + 1], pp_ps[:],
                                        mybir.AxisListType.X, _ALU.add)
                for c in a_chunks.get(g, ()):
                    nc.tensor.matmul(a_ps[:], ones_col8[:],
                                     xm[:, c * amc:(c + 1) * amc],
                                     start=(c == 0), stop=(c == nmm - 1))

            # group totals -> bias0 = ltri @ s_pg + ones_row @ excl
            stat = sm.tile([1, 2], _DT.float32)
            tot_ps = psa.tile([1, NG], _DT.float32)
            nc.tensor.matmul(tot_ps[:], ones_col[:], s_pg[:], start=True,
                             stop=True)
            incl = sm.tile([1, NG], _DT.float32)
            nc.vector.tensor_tensor_scan(
                incl[:], tot_ps[:], zeros[0:1, 0:NG], 0.0, _ALU.add,
                _ALU.bypass)
            excl = sm.tile([1, NG], _DT.float32)
            nc.vector.tensor_tensor(excl[:], incl[:], tot_ps[:],
                                    _ALU.subtract)
            nc.scalar.copy(stat[:, 1:2], incl[:, NG - 1:NG])
            bias_ps = pp.tile([P, NG], _DT.float32, name="biasps")
            nc.tensor.matmul(bias_ps[:], ltri[:], s_pg[:], start=True,
                             stop=False)
            nc.tensor.matmul(bias_ps[:], ones_row[:], excl[:], start=False,
                             stop=True)
            bias0 = sm.tile([P, NG], _DT.float32)
            nc.scalar.copy(bias0[:], bias_ps[:])
            nc.sync.dma_start(bias0_out.ap(), bias0[:])

            for c in range(13, nmm):
                nc.tensor.matmul(a_ps[:], ones_col8[:],
                                 xm[:, c * amc:(c + 1) * amc],
                                 start=(c == 0), stop=(c == nmm - 1))
            a_sb = sm.tile([1, amc], _DT.float32)
            nc.scalar.copy(a_sb[:], a_ps[:])
            nc.vector.tensor_reduce(stat[:, 0:1], a_sb[:],
                                    mybir.AxisListType.X, _ALU.add)
            nc.sync.dma_start(stat_out.ap(), stat[:])
    nc.compile()
    return nc


def _build_q():
    nc = bacc.Bacc("TRN2", target_bir_lowering=False, debug=False,
                   num_devices=NC)
    qpc_in = nc.dram_tensor("qpc", [P, FP], _DT.bfloat16,
                            kind="ExternalInput")
    bias0_in = nc.dram_tensor("bias0", [P, NG], _DT.float32,
                              kind="ExternalInput")
    w_in = nc.dram_tensor("w", [P, FP], _DT.uint8, kind="ExternalInput")
    # core offset pre-broadcast to [P, 1] on the host
    offp_in = nc.dram_tensor("offp", [P, 1], _DT.float32,
                             kind="ExternalInput")
    b_out = nc.dram_tensor("b", [1, 1], _DT.float32, kind="ExternalOutput")

    with tile.TileContext(nc) as tc:
        with (
            tc.tile_pool(name="wk", bufs=2) as wk,
            tc.tile_pool(name="sm", bufs=1) as sm,
            tc.tile_pool(name="ps", bufs=1, space="PSUM") as ps,
        ):
            # few, large input DMAs: desc-gen serializes on the sync queue
            bias0 = sm.tile([P, NG], _DT.float32)
            nc.sync.dma_start(bias0[:], bias0_in.ap())
            offp = sm.tile([P, 1], _DT.float32)
            nc.sync.dma_start(offp[:], offp_in.ap())
            qpc = sm.tile([P, FP], _DT.bfloat16, name="qpc")
            nc.sync.dma_start(qpc[:, 0:768], qpc_in.ap()[:, 0:768])
            w = sm.tile([P, FP], _DT.uint8)
            nc.sync.dma_start(w[:], w_in.ap())
            nc.sync.dma_start(qpc[:, 768:1792], qpc_in.ap()[:, 768:1792])
            nc.sync.dma_start(qpc[:, 1792:FP], qpc_in.ap()[:, 1792:FP])

            ones_col = sm.tile([P, 1], _DT.float32)
            nc.gpsimd.memset(ones_col[:], 1.0)
            # preload the Ln activation table off the critical path
            dummy = sm.tile([1, 1], _DT.float32)
            nc.gpsimd.memset(dummy[:], 1.0)
            nc.scalar.activation(dummy[:], dummy[:], _ACT.Ln)

            # bias = bias0 + core offset (per-partition scalar operand)
            bias = sm.tile([P, NG], _DT.float32)
            nc.vector.tensor_scalar(bias[:], bias0[:], offp[:, 0:1], 0.0,
                                    _ALU.add, _ALU.add)

            # ln(q + bias) per group as its DMA lands, stt trails on Vector
            acc_b = sm.tile([P, NG], _DT.float32)
            for g, (st, wd) in enumerate(GROUPS):
                sl = slice(st, st + wd)
                lnq = wk.tile([P, wd], _DT.bfloat16, name=f"lnq{g}")
                nc.scalar.activation(lnq[:], qpc[:, sl], _ACT.Ln,
                                     bias=bias[:, g:g + 1])
                junk = wk.tile([P, wd], _DT.bfloat16, name=f"junk{g}")
                nc.vector.scalar_tensor_tensor(
                    junk[:], w[:, sl], 0.0, lnq[:], _ALU.bypass,
                    _ALU.mult, accum_out=acc_b[:, g:g + 1])

            b_p = sm.tile([P, 1], _DT.float32)
            nc.vector.tensor_reduce(b_p[:], acc_b[:], mybir.AxisListType.X,
                                    _ALU.add)
            b_ps = ps.tile([1, 1], _DT.float32)
            nc.tensor.matmul(b_ps[:], b_p[:], ones_col[:], start=True,
                             stop=True)
            b_sb = sm.tile([1, 1], _DT.float32)
            nc.scalar.copy(b_sb[:], b_ps[:])
            nc.sync.dma_start(b_out.ap(), b_sb[:])
    nc.compile()
    return nc


_CACHE = {}


def _get(name, builder):
    if name not in _CACHE:
        _CACHE[name] = builder()
    return _CACHE[name]


def _prepare(x, times, events):
    import ml_dtypes

    f8 = ml_dtypes.float8_e4m3fn
    x = np.asarray(x, dtype=np.float32)
    times = np.asarray(times, dtype=np.int32)
    events = np.asarray(events, dtype=np.int32)
    assert x.shape == (N,)

    order = np.argsort(-times)           # descending time; tie order irrelevant
    xs = x[order]
    ts = times[order]
    ev = events[order].astype(bool)

    # segments = runs of equal times
    is_end = np.empty(N, dtype=bool)
    np.not_equal(ts[:-1], ts[1:], out=is_end[:-1])
    is_end[-1] = True
    ends = np.flatnonzero(is_end)
    starts = np.empty_like(ends)
    starts[0] = 0
    starts[1:] = ends[:-1] + 1
    seg_len = np.diff(np.append(starts, N))
    seg_ev = np.add.reduceat(ev.astype(np.int64), starts)
    assert seg_ev.max() < 256

    is_start = np.empty(N, dtype=bool)
    is_start[0] = True
    is_start[1:] = is_end[:-1]
    seg_id = np.cumsum(is_start) - 1
    off_in_seg = np.arange(N, dtype=np.int64) - starts[seg_id]

    pieces_per_seg = (seg_len + SW - 1) // SW
    piece_base = np.concatenate([[0], np.cumsum(pieces_per_seg)[:-1]])
    n_pieces = int(piece_base[-1] + pieces_per_seg[-1])
    assert n_pieces <= NC * PPC, (n_pieces, NC * PPC)
    per_core = -(-n_pieces // NC)

    l = piece_base[seg_id] + off_in_seg // SW
    slot = off_in_seg % SW
    c = l // per_core
    lp = l % per_core
    p = lp // FP
    f = lp % FP
    v = p // PB
    k = SW * (p % PB) + slot

    # group of piece column f, and the slot-column inside the group block
    gstarts = np.array([st for st, _ in GROUPS], dtype=np.int64)
    gwidths = np.array([wd for _, wd in GROUPS], dtype=np.int64)
    g = np.searchsorted(gstarts, f, side="right") - 1
    col = SW * gstarts[g] + v * gwidths[g] + (f - gstarts[g])

    # xp[c, k, col] = xs
    xp = np.full(NC * P * SLOTF, PAD, dtype=f8)
    dest = (c * P + k) * SLOTF + col
    xp[dest] = xs.astype(f8)
    xp = xp.reshape(NC, P, SLOTF)

    # w over pieces
    w = np.zeros(NC * PPC, dtype=np.uint8)
    last_piece = piece_base + pieces_per_seg - 1
    w[(last_piece // per_core) * PPC + last_piece % per_core] = seg_ev
    w = w.reshape(NC, P, FP)

    # compacted event-x stream, assigned to the core owning the record
    ev_pos = np.flatnonzero(ev)
    ev_core = c[ev_pos]
    xm = np.zeros((NC, P * XMF), dtype=f8)
    for cc in range(NC):
        vals = xs[ev_pos[ev_core == cc]]
        assert len(vals) <= P * XMF, (cc, len(vals))
        xm[cc, :len(vals)] = vals.astype(f8)
    xm = xm.reshape(NC, P, XMF)

    in_p = [{"xp": xp[cc], "xm": xm[cc]} for cc in range(NC)]
    w_per_core = [w[cc] for cc in range(NC)]
    return in_p, w_per_core


LAST_EXEC_NS = {}


def kernel(x, times, events):
    in_p, w_per_core = _prepare(x, times, events)
    core_ids = list(range(NC))
    trace = bool(int(os.environ.get("BASS_COX_TRACE", "0")))

    nc_p = _get("p", _build_p)
    res_p = run_bass_kernel_spmd(nc_p, in_p, core_ids=core_ids, trace=trace)

    tots = np.array([res_p.results[cc]["stat"][0, 1] for cc in range(NC)],
                    dtype=np.float64)
    offs = np.cumsum(tots) - tots
    a_tot = float(sum(res_p.results[cc]["stat"][0, 0] for cc in range(NC)))

    nc_q = _get("q", _build_q)
    in_q = []
    for cc in range(NC):
        in_q.append({
            "qpc": res_p.results[cc]["qpc"],
            "bias0": res_p.results[cc]["bias0"],
            "w": w_per_core[cc],
            "offp": np.full((P, 1), offs[cc], dtype=np.float32),
        })
    res_q = run_bass_kernel_spmd(nc_q, in_q, core_ids=core_ids, trace=trace)

    LAST_EXEC_NS.clear()
    LAST_EXEC_NS["p"] = res_p.exec_time_ns
    LAST_EXEC_NS["q"] = res_q.exec_time_ns

    b_tot = float(sum(res_q.results[cc]["b"][0, 0] for cc in range(NC)))
    loss = np.sqrt((b_tot - a_tot) / N)
    return np.float32(loss)
